# revision 11
# baseline (speedup 1.0000x reference)
"""AttentionFusion kernel for 8 Trainium2 NeuronCores (v6).

Reference computation (B=2, C=256, H=W=64, N=8192 tokens = 2 modalities x 4096):
    x    = concat(flat(feat0), flat(feat1))        # [B, N, C]
    Q,K,V = x @ W{q,k,v}.T + b{q,k,v}
    attn = softmax(Q @ K.T / 16)
    out  = (attn @ V) @ Wo.T + bo                  # [B, N, C]
    out  = mean over modalities -> [B, HW, C] -> [B, C, H, W]

Sharding: 8 cores = (2 batches) x (4 query groups). Core (b, g) computes
queries {g*1024..(g+1)*1024} of each modality (2048 rows) for batch b, with
full K/V (8192 tokens) computed locally. The modality mean pairs rows within
a core, so there is no cross-core communication at all.

v6 (from v5 @197.8us, v4 @229.4us): ACT (exp) is the hard floor -- 1
elem/cycle/lane @1.2GHz, 131072 exps/lane = ~109us + per-instruction
overhead; every other engine must stay below that pace. Measured matmul
reality: a 512-col matmul costs ~378ns wall (~225ns back-to-back) whether
fp8-DR or fp16 -- DR's win is halving instruction count by contracting
256/instr.

  - whole attention O-path in fp8 DoubleRow: exp emits p8 = fp8e4(
    exp(S/16 - 2)) ([128,1024] per kt-pair, the -2 cancels in softmax and
    keeps fp8 in range); O contracts V8-pairs (fp8, projected from fp16
    inputs). Host sim: 1.35e-2 global L2 (gate 2e-2, v4 was 1.25e-2).
  - Q and K projections folded into one: G = x_q @ (Wq.T Wk) + bq Wk on
    the query side (wqk precomputed on host), so S^T = x8.T DR G8 with no
    Q or K ever materialized. One projection phase instead of two.
  - per kt-pair: GpSimd computes t16 = p8[0]+p8[1] (stt, ~0.8us on the
    otherwise idle Q7s), DVE accumulates sa += t16 (fp16 2x mode, ~430ns).
    Denominators per chunk via the all-2.0 stationary matmul broadcast +
    reciprocal_approx_fast (bc = 0.5/sum; the 0.5 is the modality mean).
  - PSUM: S-pair pool 2x[128,2,512] (4) + O accum 2x[128,512] (2) +
    vp/bc pool 2x[128,512] (2) = 8 banks.
  - V~ (Wo@Wv folded) fp16 matmuls: 10 units pre-issued during the DMA
    wait, the rest one per pair-slot through chunk 0; evac = one strided
    DVE cast into the DR stationary layout V8[:, parity, pair*C+c].
  - single-descriptor DMAs (x8 and each x j-tile via rearranged source
    APs) to cut the ~600ns/DMA SP serialization at startup.
"""

import numpy as np

B, C, H, W = 2, 256, 64, 64
HW = H * W            # 4096
NTOK = 2 * HW         # 8192 tokens per batch (2 modalities)
NQ = 2048             # q columns per core
P = 128
KT = NTOK // P        # 64 k-tiles
NPAIR = KT // 2       # 32 kt-pairs
JT = NTOK // 1024     # 8 x-tiles of 1024 tokens
QCH = 512             # q-chunk width (1 PSUM bank)
NCH = NQ // QCH       # 4 q-chunks per core
NCORES = 8
NPRE = 10             # V units issued before attention starts

_compiled = {}


def _build():
    import concourse.bass as bass  # noqa: F401
    import concourse.mybir as mybir
    from concourse import bacc
    from concourse.tile import TileContext

    f32 = mybir.dt.float32
    f16 = mybir.dt.float16
    f8 = mybir.dt.float8e4
    DR = mybir.MatmulPerfMode.DoubleRow
    EXP = mybir.ActivationFunctionType.Exp
    ADD = mybir.AluOpType.add

    nc = bacc.Bacc("TRN2", target_bir_lowering=False, debug=False,
                   num_devices=NCORES)

    xT = nc.dram_tensor("xT", [C, NTOK], f16, kind="ExternalInput")
    xT8_d = nc.dram_tensor("xT8", [C, NTOK], f8, kind="ExternalInput")
    xTq = nc.dram_tensor("xTq", [C, NQ], f16, kind="ExternalInput")
    # wcat = [wqk | wvT] along the output axis
    wcat_d = nc.dram_tensor("wcat", [C, 2 * C], f16, kind="ExternalInput")
    # bcat = [bg | bo_eff] columns
    bcat_d = nc.dram_tensor("bcat", [C, 2], f32, kind="ExternalInput")
    out_d = nc.dram_tensor("out", [C, 1024], f32, kind="ExternalOutput")

    # [ (h p) n -> p h n ] views: one DMA descriptor per h-merged tile
    xT_v = xT.ap().rearrange("(h p) n -> p h n", h=2)
    xT8_v = xT8_d.ap().rearrange("(h p) n -> p h n", h=2)
    xTq_v = xTq.ap().rearrange("(h p) n -> p h n", h=2)

    with TileContext(nc) as tc:
        with tc.tile_pool(name="const", bufs=1) as cpool, \
             tc.tile_pool(name="kTp", bufs=1) as kTp, \
             tc.tile_pool(name="qTp", bufs=1) as qTp, \
             tc.tile_pool(name="Vp", bufs=1) as Vp:

            # scratch for PE warmup during the initial DMA wait
            wsc = cpool.tile([P, P], f16, tag="wsc")
            nc.vector.memset(wsc[:], 0.0)
            # all-2.0 stationary: one matmul broadcasts 2*colsum(sa) to
            # all 128 partitions -> reciprocal gives 0.5/sums directly
            twos = cpool.tile([P, P], f16, tag="twos")
            nc.vector.memset(twos[:], 2.0)
            # exp bias column: exp(S/16 - 2) keeps p inside fp8e4 range
            neg2 = cpool.tile([P, 1], f32, tag="neg2")
            nc.vector.memset(neg2[:], -2.0)

            # weights: [wqk | wvT] halves on partitions
            wb_sb = []
            for h in range(2):
                t = cpool.tile([P, 2 * C], f16, tag=f"wb{h}")
                nc.sync.dma_start(t[:], wcat_d.ap()[h * P:(h + 1) * P, :])
                wb_sb.append(t)
            wg_sb = [wb_sb[h][:, 0:C] for h in range(2)]
            wv_sb = [wb_sb[h][:, C:2 * C] for h in range(2)]

            # q-column slab for the G projection: [P, 2, NQ] (h-halves)
            xqf = cpool.tile([P, 2, NQ], f16, tag="xqf")
            nc.sync.dma_start(xqf[:], xTq_v)

            bb_sb = []
            for h in range(2):
                t = cpool.tile([P, 2], f32, tag=f"bb{h}")
                nc.sync.dma_start(t[:], bcat_d.ap()[h * P:(h + 1) * P, :])
                bb_sb.append(t)
            bg_sb = [bb_sb[h][:, 0:1] for h in range(2)]
            bo_sb = [bb_sb[h][:, 1:2] for h in range(2)]

            # persistent activations. x8/G8 fp8e4 at O(1) scale, laid out
            # [part, c-half, col] so a DoubleRow matmul contracts all 256
            # channels in one instruction. S^T = x8.T DR G8 with
            # G = x_q @ wqk + bg (K folded to the query side).
            x8 = kTp.tile([P, 2, NTOK], f8, tag="x8", name="x8")
            G8 = qTp.tile([P, 2, NQ], f8, tag="G8", name="G8")
            # V8: [k-part, parity, pair*256 + c] fp8 -- the DR stationary
            # for the O matmul (keys kt = 2*pair + parity).
            V8 = Vp.tile([P, 2, NPAIR * C], f8, tag="V8", name="V8")

            with tc.tile_pool(name="xcp", bufs=3) as xcp, \
                 tc.tile_pool(name="sps", bufs=2, space="PSUM") as sps, \
                 tc.tile_pool(name="ops", bufs=2, space="PSUM") as ops, \
                 tc.tile_pool(name="vps", bufs=2, space="PSUM") as vps, \
                 tc.tile_pool(name="pp", bufs=4) as pp, \
                 tc.tile_pool(name="sap", bufs=4) as sap, \
                 tc.tile_pool(name="bcp", bufs=2) as bcp, \
                 tc.tile_pool(name="nnp", bufs=6) as nnp, \
                 tc.tile_pool(name="osb", bufs=2) as osb:

                # x8 first (biggest, needed by every S); then x j-tiles
                nc.sync.dma_start(x8[:], xT8_v)
                xc_tiles = {}

                def issue_xc(j):
                    t = xcp.tile([P, 2, 1024], f16, tag="xv")
                    nc.sync.dma_start(
                        t[:], xT_v[:, :, j * 1024:(j + 1) * 1024])
                    xc_tiles[j] = t

                issue_xc(0)
                issue_xc(1)

                # ---- PE warmup: matmuls (~5us cold) flip the HAM to 8/8
                # while the first DMAs are still in flight
                wps = sps.tile([P, 2, QCH], f32, tag="sp")
                for _ in range(16):
                    nc.tensor.matmul(wps[:, 0, 0:P], wsc[:], wsc[:],
                                     start=True, stop=True)

                # ---- phase G: G^T = wqk.T-contraction of x_q + bg ----
                for cph in range(2):
                    for qh in range(2):
                        qoff = qh * 1024
                        gp = sps.tile([P, 2, QCH], f32, tag="sp")
                        for hc in range(2):
                            o = gp[:, hc, :]
                            cs = slice(qoff + hc * 512, qoff + (hc + 1) * 512)
                            nc.tensor.matmul(
                                o, wg_sb[0][:, cph * P:(cph + 1) * P],
                                xqf[:, 0, cs], start=True, stop=False)
                            nc.tensor.matmul(
                                o, wg_sb[1][:, cph * P:(cph + 1) * P],
                                xqf[:, 1, cs], start=False, stop=True)
                        nc.vector.tensor_scalar(
                            G8[:, cph, qoff:qoff + 1024].rearrange(
                                "p (a q) -> p a q", a=2),
                            gp[:], bg_sb[cph], None, ADD)

                # ---- V projection units: unit u = 256 tokens = kt-pair
                # u. 4 fp16 matmuls -> vp [P, 2, 256] PSUM, one strided
                # DVE cast into the fp8 DR layout. NPRE units run before
                # attention; the rest stream one per pair slot through
                # chunk 0's PE slack. ----
                def v_unit(u):
                    j, rem = divmod(u, 4)
                    if rem == 0 and (j + 2) < JT and (j + 2) not in xc_tiles:
                        issue_xc(j + 2)
                    xc = xc_tiles[j]
                    vp = vps.tile([P, 2, 256], f32, tag="vp")
                    for tt in range(2):
                        tok = rem * 256 + tt * P
                        nc.tensor.matmul(vp[:, tt, :],
                                         xc[:, 0, tok:tok + P], wv_sb[0],
                                         start=True, stop=False)
                        nc.tensor.matmul(vp[:, tt, :],
                                         xc[:, 1, tok:tok + P], wv_sb[1],
                                         start=False, stop=True)
                    nc.vector.tensor_copy(V8[:, :, u * C:(u + 1) * C], vp[:])

                for u in range(NPRE):
                    v_unit(u)

                # ---- attention: 4 q-chunks of 512, kt in pairs ----
                def s_exp(chunk, pair):
                    """S-pair DoubleRow matmuls + one [128,1024] exp ->
                    fp8 p8 tile [P, 2, QCH]."""
                    qb = chunk * QCH
                    sp = sps.tile([P, 2, QCH], f32, tag="sp",
                                  name=f"sp{pair}")
                    for par in range(2):
                        kt = 2 * pair + par
                        nc.tensor.matmul(
                            sp[:, par, :],
                            x8[:, :, kt * P:(kt + 1) * P],
                            G8[:, :, qb:qb + QCH],
                            start=True, stop=True, perf_mode=DR)
                    p8 = pp.tile([P, 2, QCH], f8, tag="p8",
                                 name=f"p8_{pair}")
                    nc.scalar.activation(p8[:], sp[:], EXP,
                                         bias=neg2[:], scale=1.0 / 16.0)
                    return p8

                def o_acc(o_ps, sa, p8, pair):
                    """O^T += V8-pair.T @ p8 (fp8 DR); softmax-sum side
                    split across engines: GpSimd saG += p8[0] while DVE
                    saD += p8[1] (plain tensor_tensor, fp8 -> fp16)."""
                    first, last = pair == 0, pair == NPAIR - 1
                    for ch in range(2):
                        nc.tensor.matmul(
                            o_ps[ch][:],
                            V8[:, :, pair * C + ch * P:
                               pair * C + (ch + 1) * P],
                            p8[:],
                            start=first, stop=last, perf_mode=DR)
                    saG, saD = sa
                    nc.gpsimd.tensor_add(saG[:], saG[:], p8[:, 0, :])
                    nc.vector.tensor_add(saD[:], saD[:], p8[:, 1, :])

                stash = {}
                for chunk in range(NCH):
                    o_ps = [ops.tile([P, QCH], f32, tag="op",
                                     name=f"o{chunk}_{ch}")
                            for ch in range(2)]
                    saG = sap.tile([P, QCH], f16, tag="saG")
                    saD = sap.tile([P, QCH], f16, tag="saD")
                    nc.gpsimd.memset(saG[:], 0.0)
                    nc.vector.memset(saD[:], 0.0)
                    sa = (saG, saD)

                    # software pipeline, 2 deep: PE order is
                    # S(0), S(1), O(0), S(2), O(1), ...
                    p_prev = s_exp(chunk, 0)
                    for pair in range(1, NPAIR):
                        if chunk == 0:
                            u = NPRE + pair - 1
                            if u <= NPAIR - 1:
                                v_unit(u)
                        p_cur = s_exp(chunk, pair)
                        o_acc(o_ps, sa, p_prev, pair - 1)
                        p_prev = p_cur
                    o_acc(o_ps, sa, p_prev, NPAIR - 1)

                    # bc = 0.5/sums on all 128 partitions:
                    # twos.T @ (saG + saD) = 2*sums broadcast, then 1/x
                    bc_ps = vps.tile([P, 2, 256], f32, tag="vp")
                    bcv = bc_ps[:].rearrange("p a q -> p (a q)")
                    nc.tensor.matmul(bcv, twos[:], saG[:],
                                     start=True, stop=False)
                    nc.tensor.matmul(bcv, twos[:], saD[:],
                                     start=False, stop=True)
                    bc = bcp.tile([P, QCH], f32, tag="bc")
                    nc.vector.reciprocal_approx_fast(bc[:], bcv)

                    mod, qh = chunk // 2, chunk % 2
                    if mod == 0:
                        st = []
                        for ch in range(2):
                            t = nnp.tile([P, QCH], f32, tag="nn")
                            nc.vector.tensor_mul(t[:], o_ps[ch][:], bc[:])
                            st.append(t)
                        stash[qh] = st
                    else:
                        for ch in range(2):
                            t = nnp.tile([P, QCH], f32, tag="nn")
                            nc.vector.tensor_mul(t[:], o_ps[ch][:], bc[:])
                            ot = osb.tile([P, QCH], f32, tag="os")
                            # out = (n1 + bo_eff) + n0 in one fused op
                            nc.vector.scalar_tensor_tensor(
                                ot[:], t[:], bo_sb[ch], stash[qh][ch][:],
                                ADD, ADD)
                            nc.sync.dma_start(
                                out_d.ap()[ch * P:(ch + 1) * P,
                                           qh * QCH:(qh + 1) * QCH], ot[:])

    nc.compile()
    return nc


def _get_compiled():
    if "nc" not in _compiled:
        _compiled["nc"] = _build()
    return _compiled["nc"]


def kernel(feat0, feat1, Wq, bq, Wk, bk, Wv, bv, Wo, bo):
    from concourse.bass_utils import run_bass_kernel_spmd

    feat0 = np.asarray(feat0, dtype=np.float32)
    feat1 = np.asarray(feat1, dtype=np.float32)
    Wq = np.asarray(Wq, dtype=np.float32)
    Wk = np.asarray(Wk, dtype=np.float32)
    Wv = np.asarray(Wv, dtype=np.float32)
    Wo = np.asarray(Wo, dtype=np.float32)
    bq = np.asarray(bq, dtype=np.float32)
    bv = np.asarray(bv, dtype=np.float32)
    bo = np.asarray(bo, dtype=np.float32)

    import ml_dtypes

    # Q/K fold: G = x_q @ (Wq.T Wk) + bq Wk, S = X @ G.T
    wqk = Wq.T @ Wk
    # fold output projection into V: V~ = X @ (Wo @ Wv).T
    wvT = (Wo @ Wv).T
    wcat = np.ascontiguousarray(
        np.concatenate([wqk, wvT], axis=1)).astype(np.float16)
    bg = bq @ Wk
    bo_eff = bo + Wo @ bv
    bcat = np.ascontiguousarray(np.stack([bg, bo_eff], axis=1))

    xT_all = [
        np.ascontiguousarray(
            np.concatenate([feat0[b].reshape(C, HW), feat1[b].reshape(C, HW)],
                           axis=1)).astype(np.float16)
        for b in range(B)
    ]
    xT8_all = [np.ascontiguousarray(x.astype(ml_dtypes.float8_e4m3fn))
               for x in xT_all]

    in_maps = []
    for core in range(NCORES):
        b, g = core // 4, core % 4
        cols0 = slice(g * 1024, (g + 1) * 1024)
        cols1 = slice(HW + g * 1024, HW + (g + 1) * 1024)
        xTq = np.ascontiguousarray(
            np.concatenate([xT_all[b][:, cols0], xT_all[b][:, cols1]],
                           axis=1))
        in_maps.append({
            "xT": xT_all[b], "xT8": xT8_all[b], "xTq": xTq,
            "wcat": wcat, "bcat": bcat,
        })

    global _last_in_maps
    _last_in_maps = in_maps

    nc = _get_compiled()
    res = run_bass_kernel_spmd(nc, in_maps, core_ids=list(range(NCORES)))

    full = np.empty((B, C, HW), dtype=np.float32)
    for core in range(NCORES):
        b, g = core // 4, core % 4
        full[b][:, g * 1024:(g + 1) * 1024] = res.results[core]["out"]
    return full.reshape(B, C, H, W)


# revision 18
# speedup vs baseline: 1.2269x; 1.2269x over previous
"""AttentionFusion kernel for 8 Trainium2 NeuronCores (v6).

Reference computation (B=2, C=256, H=W=64, N=8192 tokens = 2 modalities x 4096):
    x    = concat(flat(feat0), flat(feat1))        # [B, N, C]
    Q,K,V = x @ W{q,k,v}.T + b{q,k,v}
    attn = softmax(Q @ K.T / 16)
    out  = (attn @ V) @ Wo.T + bo                  # [B, N, C]
    out  = mean over modalities -> [B, HW, C] -> [B, C, H, W]

Sharding: 8 cores = (2 batches) x (4 query groups). Core (b, g) computes
queries {g*1024..(g+1)*1024} of each modality (2048 rows) for batch b, with
full K/V (8192 tokens) computed locally. The modality mean pairs rows within
a core, so there is no cross-core communication at all.

v6 (from v5 @197.8us, v4 @229.4us): ACT (exp) is the hard floor -- 1
elem/cycle/lane @1.2GHz, 131072 exps/lane = ~109us + per-instruction
overhead; every other engine must stay below that pace. Measured matmul
reality: a 512-col matmul costs ~378ns wall (~225ns back-to-back) whether
fp8-DR or fp16 -- DR's win is halving instruction count by contracting
256/instr.

  - whole attention O-path in fp8 DoubleRow: exp emits p8 = fp8e4(
    exp(S/16 - 2)) ([128,1024] per kt-pair, the -2 cancels in softmax and
    keeps fp8 in range); O contracts V8-pairs (fp8, projected from fp16
    inputs). Host sim: 1.35e-2 global L2 (gate 2e-2, v4 was 1.25e-2).
  - Q and K projections folded into one: G = x_q @ (Wq.T Wk) + bq Wk on
    the query side (wqk precomputed on host), so S^T = x8.T DR G8 with no
    Q or K ever materialized. One projection phase instead of two.
  - per kt-pair: GpSimd computes t16 = p8[0]+p8[1] (stt, ~0.8us on the
    otherwise idle Q7s), DVE accumulates sa += t16 (fp16 2x mode, ~430ns).
    Denominators per chunk via the all-2.0 stationary matmul broadcast +
    reciprocal_approx_fast (bc = 0.5/sum; the 0.5 is the modality mean).
  - PSUM: S-pair pool 2x[128,2,512] (4) + O accum 2x[128,512] (2) +
    vp/bc pool 2x[128,512] (2) = 8 banks.
  - V~ (Wo@Wv folded) fp16 matmuls: 10 units pre-issued during the DMA
    wait, the rest one per pair-slot through chunk 0; evac = one strided
    DVE cast into the DR stationary layout V8[:, parity, pair*C+c].
  - single-descriptor DMAs (x8 and each x j-tile via rearranged source
    APs) to cut the ~600ns/DMA SP serialization at startup.
"""

import numpy as np

B, C, H, W = 2, 256, 64, 64
HW = H * W            # 4096
NTOK = 2 * HW         # 8192 tokens per batch (2 modalities)
NQ = 2048             # q columns per core
P = 128
KT = NTOK // P        # 64 k-tiles
NPAIR = KT // 2       # 32 kt-pairs
JT = NTOK // 1024     # 8 x-tiles of 1024 tokens
QCH = 512             # q-chunk width (1 PSUM bank)
NCH = NQ // QCH       # 4 q-chunks per core
NCORES = 8
NPRE = 10             # V units issued before attention starts

_compiled = {}


def _build():
    import concourse.bass as bass  # noqa: F401
    import concourse.mybir as mybir
    from concourse import bacc
    from concourse.tile import TileContext

    f32 = mybir.dt.float32
    f16 = mybir.dt.float16
    f8 = mybir.dt.float8e4
    DR = mybir.MatmulPerfMode.DoubleRow
    EXP = mybir.ActivationFunctionType.Exp
    ADD = mybir.AluOpType.add

    nc = bacc.Bacc("TRN2", target_bir_lowering=False, debug=False,
                   num_devices=NCORES)

    xT = nc.dram_tensor("xT", [C, NTOK], f16, kind="ExternalInput")
    xT8_d = nc.dram_tensor("xT8", [C, NTOK], f8, kind="ExternalInput")
    xTq = nc.dram_tensor("xTq", [C, NQ], f16, kind="ExternalInput")
    # wcat = [wqk | wvT] along the output axis
    wcat_d = nc.dram_tensor("wcat", [C, 2 * C], f16, kind="ExternalInput")
    # bcat = [bg | bo_eff] columns
    bcat_d = nc.dram_tensor("bcat", [C, 2], f32, kind="ExternalInput")
    out_d = nc.dram_tensor("out", [C, 1024], f32, kind="ExternalOutput")

    # [ (h p) n -> p h n ] views: one DMA descriptor per h-merged tile
    xT_v = xT.ap().rearrange("(h p) n -> p h n", h=2)
    xT8_v = xT8_d.ap().rearrange("(h p) n -> p h n", h=2)
    xTq_v = xTq.ap().rearrange("(h p) n -> p h n", h=2)

    with TileContext(nc) as tc:
        with tc.tile_pool(name="const", bufs=1) as cpool, \
             tc.tile_pool(name="kTp", bufs=1) as kTp, \
             tc.tile_pool(name="qTp", bufs=1) as qTp, \
             tc.tile_pool(name="Vp", bufs=1) as Vp:

            # scratch for PE warmup during the initial DMA wait
            wsc = cpool.tile([P, P], f16, tag="wsc")
            nc.vector.memset(wsc[:], 0.0)
            # all-2.0 stationary: one matmul broadcasts 2*colsum(sa) to
            # all 128 partitions -> reciprocal gives 0.5/sums directly
            twos = cpool.tile([P, P], f16, tag="twos")
            nc.vector.memset(twos[:], 2.0)
            # exp bias column: exp(S/16 - 2) keeps p inside fp8e4 range
            neg2 = cpool.tile([P, 1], f32, tag="neg2")
            nc.vector.memset(neg2[:], -2.0)
            # zero column for GpSimd add-zero casts (PSUM f32 -> fp8)
            zcol = cpool.tile([P, 1], f32, tag="zcol")
            nc.vector.memset(zcol[:], 0.0)

            # weights: [wqk | wvT] halves on partitions
            wb_sb = []
            for h in range(2):
                t = cpool.tile([P, 2 * C], f16, tag=f"wb{h}")
                nc.sync.dma_start(t[:], wcat_d.ap()[h * P:(h + 1) * P, :])
                wb_sb.append(t)
            wg_sb = [wb_sb[h][:, 0:C] for h in range(2)]
            wv_sb = [wb_sb[h][:, C:2 * C] for h in range(2)]

            # q-column slab for the G projection: [P, 2, NQ] (h-halves)
            xqf = cpool.tile([P, 2, NQ], f16, tag="xqf")
            nc.sync.dma_start(xqf[:], xTq_v)

            bb_sb = []
            for h in range(2):
                t = cpool.tile([P, 2], f32, tag=f"bb{h}")
                nc.sync.dma_start(t[:], bcat_d.ap()[h * P:(h + 1) * P, :])
                bb_sb.append(t)
            bg_sb = [bb_sb[h][:, 0:1] for h in range(2)]
            bo_sb = [bb_sb[h][:, 1:2] for h in range(2)]

            # persistent activations. x8/G8 fp8e4 at O(1) scale, laid out
            # [part, c-half, col] so a DoubleRow matmul contracts all 256
            # channels in one instruction. S^T = x8.T DR G8 with
            # G = x_q @ wqk + bg (K folded to the query side).
            x8 = kTp.tile([P, 2, NTOK], f8, tag="x8", name="x8")
            G8 = qTp.tile([P, 2, NQ], f8, tag="G8", name="G8")
            # V8: [k-part, parity, pair*256 + c] fp8 -- the DR stationary
            # for the O matmul (keys kt = 2*pair + parity).
            V8 = Vp.tile([P, 2, NPAIR * C], f8, tag="V8", name="V8")

            with tc.tile_pool(name="xcp", bufs=3) as xcp, \
                 tc.tile_pool(name="sps", bufs=2, space="PSUM") as sps, \
                 tc.tile_pool(name="ops", bufs=2, space="PSUM") as ops, \
                 tc.tile_pool(name="vps", bufs=2, space="PSUM") as vps, \
                 tc.tile_pool(name="pp", bufs=4) as pp, \
                 tc.tile_pool(name="sap", bufs=2) as sap, \
                 tc.tile_pool(name="tmp", bufs=3) as tmpp, \
                 tc.tile_pool(name="bcp", bufs=2) as bcp, \
                 tc.tile_pool(name="nnp", bufs=6) as nnp, \
                 tc.tile_pool(name="osb", bufs=2) as osb:

                # x8 first (biggest, needed by every S); then x j-tiles
                nc.sync.dma_start(x8[:], xT8_v)
                xc_tiles = {}

                def issue_xc(j):
                    t = xcp.tile([P, 2, 1024], f16, tag="xv")
                    nc.sync.dma_start(
                        t[:], xT_v[:, :, j * 1024:(j + 1) * 1024])
                    xc_tiles[j] = t

                issue_xc(0)
                issue_xc(1)

                # ---- PE warmup: matmuls (~5us cold) flip the HAM to 8/8
                # while the first DMAs are still in flight
                wps = sps.tile([P, 2, QCH], f32, tag="sp")
                for _ in range(16):
                    nc.tensor.matmul(wps[:, 0, 0:P], wsc[:], wsc[:],
                                     start=True, stop=True)

                # ---- phase G: G^T = wqk.T-contraction of x_q + bg ----
                for cph in range(2):
                    for qh in range(2):
                        qoff = qh * 1024
                        gp = sps.tile([P, 2, QCH], f32, tag="sp")
                        for hc in range(2):
                            o = gp[:, hc, :]
                            cs = slice(qoff + hc * 512, qoff + (hc + 1) * 512)
                            nc.tensor.matmul(
                                o, wg_sb[0][:, cph * P:(cph + 1) * P],
                                xqf[:, 0, cs], start=True, stop=False)
                            nc.tensor.matmul(
                                o, wg_sb[1][:, cph * P:(cph + 1) * P],
                                xqf[:, 1, cs], start=False, stop=True)
                        nc.vector.tensor_scalar(
                            G8[:, cph, qoff:qoff + 1024].rearrange(
                                "p (a q) -> p a q", a=2),
                            gp[:], bg_sb[cph], None, ADD)

                # ---- V projection units: unit u = 256 tokens = kt-pair
                # u. 4 fp16 matmuls -> vp [P, 2, 256] PSUM, one strided
                # DVE cast into the fp8 DR layout. NPRE units run before
                # attention; the rest stream one per pair slot through
                # chunk 0's PE slack. ----
                def v_unit(u):
                    j, rem = divmod(u, 4)
                    if rem == 0 and (j + 2) < JT and (j + 2) not in xc_tiles:
                        issue_xc(j + 2)
                    xc = xc_tiles[j]
                    vp = vps.tile([P, 2, 256], f32, tag="vp")
                    for tt in range(2):
                        tok = rem * 256 + tt * P
                        nc.tensor.matmul(vp[:, tt, :],
                                         xc[:, 0, tok:tok + P], wv_sb[0],
                                         start=True, stop=False)
                        nc.tensor.matmul(vp[:, tt, :],
                                         xc[:, 1, tok:tok + P], wv_sb[1],
                                         start=False, stop=True)
                    nc.vector.tensor_copy(V8[:, :, u * C:(u + 1) * C], vp[:])

                for u in range(NPRE):
                    v_unit(u)

                # ---- attention: 4 q-chunks of 512, kt in pairs ----
                def s_exp(chunk, pair):
                    """S-pair DoubleRow matmuls + one [128,1024] exp ->
                    fp8 p8 tile [P, 2, QCH]."""
                    qb = chunk * QCH
                    sp = sps.tile([P, 2, QCH], f32, tag="sp",
                                  name=f"sp{pair}")
                    for par in range(2):
                        kt = 2 * pair + par
                        nc.tensor.matmul(
                            sp[:, par, :],
                            x8[:, :, kt * P:(kt + 1) * P],
                            G8[:, :, qb:qb + QCH],
                            start=True, stop=True, perf_mode=DR)
                    p8 = pp.tile([P, 2, QCH], f8, tag="p8",
                                 name=f"p8_{pair}")
                    nc.scalar.activation(p8[:], sp[:], EXP,
                                         bias=neg2[:], scale=1.0 / 16.0)
                    return p8

                def o_acc(o_ps, sa, p8, pair):
                    """O^T += V8-pair.T @ p8 (fp8 DR); softmax sums on
                    DVE: t16 = p8[0]+p8[1] (stt, 1x), sa += t16 (2x)."""
                    first, last = pair == 0, pair == NPAIR - 1
                    for ch in range(2):
                        nc.tensor.matmul(
                            o_ps[ch][:],
                            V8[:, :, pair * C + ch * P:
                               pair * C + (ch + 1) * P],
                            p8[:],
                            start=first, stop=last, perf_mode=DR)
                    t16 = tmpp.tile([P, QCH], f16, tag="t16")
                    nc.vector.scalar_tensor_tensor(
                        t16[:], p8[:, 0, :], 0.0, p8[:, 1, :], ADD, ADD)
                    nc.vector.tensor_add(sa[:], sa[:], t16[:])

                stash = {}
                for chunk in range(NCH):
                    o_ps = [ops.tile([P, QCH], f32, tag="op",
                                     name=f"o{chunk}_{ch}")
                            for ch in range(2)]
                    sa = sap.tile([P, QCH], f16, tag="sa")
                    nc.vector.memset(sa[:], 0.0)

                    # software pipeline, 2 deep: PE order is
                    # S(0), S(1), O(0), S(2), O(1), ...
                    p_prev = s_exp(chunk, 0)
                    for pair in range(1, NPAIR):
                        if chunk == 0:
                            u = NPRE + pair - 1
                            if u <= NPAIR - 1:
                                v_unit(u)
                        p_cur = s_exp(chunk, pair)
                        o_acc(o_ps, sa, p_prev, pair - 1)
                        p_prev = p_cur
                    o_acc(o_ps, sa, p_prev, NPAIR - 1)

                    # bc = 0.5/sums on all 128 partitions:
                    # twos.T @ sa = 2*sums broadcast, then 1/x
                    bc_ps = vps.tile([P, 2, 256], f32, tag="vp")
                    bcv = bc_ps[:].rearrange("p a q -> p (a q)")
                    nc.tensor.matmul(bcv, twos[:], sa[:],
                                     start=True, stop=True)
                    bc = bcp.tile([P, QCH], f32, tag="bc")
                    nc.vector.reciprocal_approx_fast(bc[:], bcv)

                    mod, qh = chunk // 2, chunk % 2
                    if mod == 0:
                        st = []
                        for ch in range(2):
                            t = nnp.tile([P, QCH], f32, tag="nn")
                            nc.vector.tensor_mul(t[:], o_ps[ch][:], bc[:])
                            st.append(t)
                        stash[qh] = st
                    else:
                        for ch in range(2):
                            t = nnp.tile([P, QCH], f32, tag="nn")
                            nc.vector.tensor_mul(t[:], o_ps[ch][:], bc[:])
                            ot = osb.tile([P, QCH], f32, tag="os")
                            # out = (n1 + bo_eff) + n0 in one fused op
                            nc.vector.scalar_tensor_tensor(
                                ot[:], t[:], bo_sb[ch], stash[qh][ch][:],
                                ADD, ADD)
                            nc.sync.dma_start(
                                out_d.ap()[ch * P:(ch + 1) * P,
                                           qh * QCH:(qh + 1) * QCH], ot[:])

    nc.compile()
    return nc


def _get_compiled():
    if "nc" not in _compiled:
        _compiled["nc"] = _build()
    return _compiled["nc"]


def kernel(feat0, feat1, Wq, bq, Wk, bk, Wv, bv, Wo, bo):
    from concourse.bass_utils import run_bass_kernel_spmd

    feat0 = np.asarray(feat0, dtype=np.float32)
    feat1 = np.asarray(feat1, dtype=np.float32)
    Wq = np.asarray(Wq, dtype=np.float32)
    Wk = np.asarray(Wk, dtype=np.float32)
    Wv = np.asarray(Wv, dtype=np.float32)
    Wo = np.asarray(Wo, dtype=np.float32)
    bq = np.asarray(bq, dtype=np.float32)
    bv = np.asarray(bv, dtype=np.float32)
    bo = np.asarray(bo, dtype=np.float32)

    import ml_dtypes

    # Q/K fold: G = x_q @ (Wq.T Wk) + bq Wk, S = X @ G.T
    wqk = Wq.T @ Wk
    # fold output projection into V: V~ = X @ (Wo @ Wv).T
    wvT = (Wo @ Wv).T
    wcat = np.ascontiguousarray(
        np.concatenate([wqk, wvT], axis=1)).astype(np.float16)
    bg = bq @ Wk
    bo_eff = bo + Wo @ bv
    bcat = np.ascontiguousarray(np.stack([bg, bo_eff], axis=1))

    xT_all = [
        np.ascontiguousarray(
            np.concatenate([feat0[b].reshape(C, HW), feat1[b].reshape(C, HW)],
                           axis=1)).astype(np.float16)
        for b in range(B)
    ]
    xT8_all = [np.ascontiguousarray(x.astype(ml_dtypes.float8_e4m3fn))
               for x in xT_all]

    in_maps = []
    for core in range(NCORES):
        b, g = core // 4, core % 4
        cols0 = slice(g * 1024, (g + 1) * 1024)
        cols1 = slice(HW + g * 1024, HW + (g + 1) * 1024)
        xTq = np.ascontiguousarray(
            np.concatenate([xT_all[b][:, cols0], xT_all[b][:, cols1]],
                           axis=1))
        in_maps.append({
            "xT": xT_all[b], "xT8": xT8_all[b], "xTq": xTq,
            "wcat": wcat, "bcat": bcat,
        })

    global _last_in_maps
    _last_in_maps = in_maps

    nc = _get_compiled()
    res = run_bass_kernel_spmd(nc, in_maps, core_ids=list(range(NCORES)))

    full = np.empty((B, C, HW), dtype=np.float32)
    for core in range(NCORES):
        b, g = core // 4, core % 4
        full[b][:, g * 1024:(g + 1) * 1024] = res.results[core]["out"]
    return full.reshape(B, C, H, W)
